# revision 30
# baseline (speedup 1.0000x reference)
"""AtomicConvolution Trainium2 kernel (8 NeuronCores, SPMD, no collectives).

Sharding: N-shard. Core r handles atoms [256r, 256r+256) for ALL 16 batches.
The X coordinate table (tiny) is replicated per core, so the neighbor gather
is core-local and the batch-norm moments over axis 0 (batch) are core-local
too (each core holds all 16 batches for its atoms). No cross-core traffic.

The GPSIMD ap_gather is the bottleneck at ~23-32ns/group-index (device-state
dependent), and measurement shows the rate is pinned by the SBUF random-read
path itself: the native Pool-engine IndirectCopy, f16 tables, small tables,
d>1, and any instruction chunking all land at the same per-index cost, and
the DMA-engine gather (InstDMAGatherAnt, mlp library - it does work on this
firmware, the old crash note is stale) is WORSE: its Q7 descriptor
generation costs ~8ns/idx on the same sequencer that runs ap_gather, so
hybrids lose. The only real lever is gathering FEWER slots:

  - Z-FILTER (the big win): Nbrs_Z is drawn from [0,10) but only types
    {1,6,7,8} match a mask plane, so ~60% of slots are multiplied by zero
    in every output. The host keeps matching slots first (stable argsort of
    ~isin) and truncates each (b,n) list; kept non-matching slots are
    harmless fillers. Exactly correct, not an approximation.
  - ATOM SORT + PER-CHUNK WIDTHS: each core's atoms are sorted by their
    worst-case (max over 16 batches) matching count and laid out so chunk k
    covers sorted ranks [32k, 32k+32); chunk widths mps[k] are the
    cross-core envelope of the per-chunk maxima (one compiled graph runs on
    all 8 cores), rounded up to 2 (even widths keep f16 2x DVE modes). For
    the fixed harness dataset this gives [30,32,32,34,34,34,36 |
    36,38,38,48] = ~30% fewer gather indices than uniform 48 and ~47% fewer
    than the unfiltered 64. The permutation is undone on host in
    assemble_output.

Per-core pipeline per chunk (stage s hides under gather s+1):
  - gather table tbl[16g+c, beta*2048 + j] = plane c of X[2g+beta, j]
    (partition-group g owns batches {2g, 2g+1}; c=0,1,2 -> x,y,z; c=3 -> 0)
  - ap_gather (d=1, 7 main chunks of 64*mp idx/group + the last chunk as
    four 16*mp quarters so only a tiny stage trails the final gather;
    2-deep output ring) pulls all 16 channels per index -> x,y,z per read
  - dx^2 = (gathered - centers)^2 in ONE fused custom-DVE op (SQDIFF,
    registered at import into dve_ops.OPS; one call per (beta, j) since
    custom-DVE APs are rank<=3); R^2 = PE ones-block matmul reducing the 4
    channels; R = exp(0.5 ln R^2) on ACT (stays in the ln/exp table set);
    one SBUF->SBUF DMA compacts R to [128, tf] with partition p = 8b + nb,
    free = per-chunk (j, m) blocks at ci/16-prefix offsets (gidx column
    offset == Rt column offset)
  - rsf_l = exp(-re(R-rs)^2) * 0.5*(cos(pi R/rc)+1) * [R<=rc]:
      u = Square(R - rs) (ACT); K' = Exp(-re*u + ln 0.5) (ACT)
      cos via degree-5 polynomial in y = Relu(pi - pi R/rc)^2 (ACT Relu +
      ACT Square + TWO fused Horner custom-DVE ops POLY3/POLY3B; exact
      cutoff, max err 2.4e-6)
      rsf = (1 - cos)*K' via one stock scalar_tensor_tensor
  - per l: ONE broadcast multiply against the packed 4-type mask + ONE
    segmented reduce into a transposed sym accumulator; the R path stays
    f32 (exp(-re(R-rs)^2) is brutally sensitive to R error) but the
    post-exp path runs in fp16 (2x DVE modes, less SBUF traffic: measured
    DVE SBUF traffic during gathers costs ~1:1 in gather slowdown, while
    ACT traffic is free - keep DVE lean, ACT can soak work)
  - BN over the 16 batches: PE stride-8 partition reductions + broadcasts;
    the final multiply writes through a strided AP to restore (ns, a*12+l)
    output order. Stages are emitted BEFORE the next chunk's DVE ops so the
    in-order DVE queue never head-of-line blocks on the gather.

End-to-end rel err ~1.7e-3 vs the 2e-2 gate (identical to the unfiltered
baseline - the filter/sort/widths change is exact). Measured on-device
(median of interleaved reps=257 pair differences; wall-clock pairing drifts
by tens of ms mid-session so only adjacent same-state comparisons count):
baseline 1.32ms -> Z-filter mp=48 + fused DVE ops: -407us/rep -> adaptive
per-chunk widths: -227us/rep more (~603-642us absolute); roundup2 widths
and a partition-split tbl load (the 2MB table DMA gates the first gather;
halves go on both HWDGE queues) shave a further few percent of indices and
head latency.
"""
import sys

if '/opt/trn_rl_repo' not in sys.path:
    sys.path.insert(0, '/opt/trn_rl_repo')

import math
import numpy as np

import concourse.bacc as bacc
import concourse.bass as bass
import concourse.mybir as mybir
from concourse import library_config
from concourse.tile import TileContext

# Steer the act-table-load pass: every ACT func this kernel uses (exp, ln,
# square, relu) lives in natural_log_exp_and_others, but the insertion pass
# maps each func to the FIRST set containing it, so ln<->exp would reload the
# table on every switch. Strip those funcs from all other sets (set order and
# ids are unchanged) so everything resolves to the one shared set and the
# single load hoists out of the loop.
import concourse.hw_specs as _hw_specs
if not getattr(_hw_specs, "_act_tbl_patched", False):
    _orig_gat = _hw_specs.get_activation_tables

    def _gat_one_set(arch):
        tabs = _orig_gat(arch)
        keep = "natural_log_exp_and_others"
        if keep not in tabs:
            return tabs
        shared = tabs[keep]
        return {name: (fs if name == keep else fs - shared)
                for name, fs in tabs.items()}

    _hw_specs.get_activation_tables = _gat_one_set
    _hw_specs._act_tbl_patched = True
    bacc.get_activation_tables = _gat_one_set

# Give the Tile scheduler realistic ap_gather timing. The stock cost model
# rates a 4096-idx gather at ~5.7us; measured hardware is ~93us. With the
# stock number the static schedule believes gathers finish instantly and
# orders the next chunk's DVE ops ahead of the previous R-stage, so the
# in-order DVE queue head-of-line blocks on the gather and ~120us of stage
# work drains serially after the last gather. 0.0366 calibrates the model
# to 22.8ns/group-idx. Must run before the first build in the process (the
# rust side caches the spec in a OnceLock).
if "APGather" not in _hw_specs.TRN2Spec.GPSIMD_IMPL_EFFICIENCY:
    for _k in ("APGather", "InstAPGather", "ISA"):
        _hw_specs.TRN2Spec.GPSIMD_IMPL_EFFICIENCY[_k] = 0.0366

F32 = mybir.dt.float32
F16 = mybir.dt.float16
BF16 = mybir.dt.bfloat16
I16 = mybir.dt.int16
AF = mybir.ActivationFunctionType
ALU = mybir.AluOpType

# ---- runtime-registered fused DVE ops (fewer passes -> less SBUF traffic
# concurrent with the Q7 gather's random reads, which measurement shows is
# nearly 1:1 additive with gather time).
import numpy as _np
import concourse.dve_ops as _dve_ops
from concourse.dve_spec import (Spec as _Spec, Src0 as _S0, Src1 as _S1,
                                C0 as _DC0, C1 as _DC1, C2 as _DC2,
                                sq as _sq, lower as _dve_lower,
                                _has_src1 as _dve_has_src1)
from concourse.dve_uop import DveOpSpec as _DveOpSpec


def _reg_dve(name, spec):
    for o in _dve_ops.OPS:
        if o.name == name:
            return o
    row = max(_dve_ops._SUB_OPCODE_FOR_NAME.values()) + 1
    assert row < 0x20
    tmp = _DveOpSpec(name=name, opcode=row, uops=_dve_lower(spec, ver="v3"),
                     rd1_en=_dve_has_src1(spec))
    op = _dve_ops.DveOp(name, spec, subdim=False,
                        uops_sha={"v3": tmp.sha("v3")})
    _dve_ops.OPS.append(op)
    _dve_ops.CUSTOM_DVE_SPECS[name] = spec
    _dve_ops._SUB_OPCODE_FOR_NAME[name] = row
    return op


# out = (in0 - in1)^2   (dx then dx*dx in one pass)
SQDIFF = _reg_dve("SQDIFF_ANT", _Spec(
    body=_sq(_S0 - _S1),
    reference=lambda in0, in1, s0, s1, imm2:
        ((in0.astype(_np.float32) - in1) ** 2).astype(_np.float32)))
# out = (s0*in0 + s1)*in0 + imm2   (Horner head of the cos polynomial)
POLY3 = _reg_dve("POLY3_ANT", _Spec(
    body=(_DC0 * _S0 + _DC1) * _S0 + _DC2,
    reference=lambda in0, in1, s0, s1, imm2:
        ((s0 * in0.astype(_np.float32) + s1) * in0 + imm2).astype(_np.float32)))
# out = ((in0*in1 + s0)*in1 + s1)*in1 + imm2   (Horner tail)
POLY3B = _reg_dve("POLY3B_ANT", _Spec(
    body=((_S0 * _S1 + _DC0) * _S1 + _DC1) * _S1 + _DC2,
    reference=lambda in0, in1, s0, s1, imm2:
        (((in0.astype(_np.float32) * in1 + s0) * in1 + s1) * in1 + imm2
         ).astype(_np.float32)))

P = 128
B, N, M, L, A = 16, 2048, 64, 12, 4
NSH = N // 8                 # atoms per core = 256
NFEAT = A * L                # 48
OUTF = 32 * NFEAT            # 1536 output cols per partition
ATOM_TYPES = (1, 6, 7, 8)
BN_EPS = 1e-3
PI = math.pi
GCH_BUFS = 2
# cos(x) on [0, pi] as a degree-5 polynomial in y = x^2 (max err 2.4e-6)
COS_B = (0.9999994437, -0.4999955817, 0.0416610328, -0.0013862747,
         2.42532e-05, -2.219e-07)


def build_nc(rc_v, rs_v, re_v, reps=None, ablate=(), mp=64, mps=None):
    """Build the per-core graph. rc/rs/re are baked in as immediates.
    reps: if set, wrap the whole body in a HW For_i loop (for benchmarking).
    ablate: subset of {"gather","prod","mm","quarter"} to skip (profiling).
    mps: per-chunk slot widths after the host's Z-filter, 11 entries (7 main
    chunks of 4 ns + 4 quarter chunks of 1 ns). Only slots whose Z matches
    one of the 4 atom types contribute to the output, so the host compacts
    each neighbor list to the matching slots (padded with non-matching
    ones, which the type masks zero out exactly). The host additionally
    sorts each core's atoms by worst-case matching count (undone at output
    assembly) so early chunks get narrower widths. mp is the uniform
    fallback when mps is None."""
    ablate = set(ablate)
    if mps is None:
        mps = [mp] * 11
    rc_v = [float(x) for x in rc_v]
    rs_v = [float(x) for x in rs_v]
    re_v = [float(x) for x in re_v]
    rc_groups = {}
    for l, v in enumerate(rc_v):
        rc_groups.setdefault(v, []).append(l)
    rc_list = list(rc_groups.keys())
    rcg_of_l = {}
    for gi, v in enumerate(rc_list):
        for l in rc_groups[v]:
            rcg_of_l[l] = gi

    tf = sum(4 * m for m in mps[:7]) + sum(mps[7:])  # compacted R free size
    nc = bacc.Bacc()
    tbl_in = nc.declare_dram_parameter("tbl", [P, 2 * N], F32, isOutput=False)
    gidx_in = nc.declare_dram_parameter("gidx", [P, tf], I16, isOutput=False)
    cen_in = nc.declare_dram_parameter("cen", [P, 2 * NSH], F32, isOutput=False)
    zc_in = nc.declare_dram_parameter("zc", [P, 4 * tf], F16,
                                      isOutput=False)
    wq_in = nc.declare_dram_parameter("wq", [P, 8], F32, isOutput=False)
    bnred_in = nc.declare_dram_parameter("bnred", [P, 8], F32, isOutput=False)
    bnbc_in = nc.declare_dram_parameter("bnbc", [8, P], F32, isOutput=False)
    cb_in = nc.declare_dram_parameter("cbias", [P, 32], F32, isOutput=False)
    out_ext = nc.declare_dram_parameter("out", [P, OUTF], F32, isOutput=True)

    import contextlib
    with TileContext(nc) as tc:
        with tc.tile_pool(name="sbuf", bufs=1) as pool, \
             tc.tile_pool(name="psum", bufs=1, space="PSUM") as psum:
            nc.gpsimd.load_library(library_config.ap_gather)
            loop_cm = tc.For_i(0, reps, 1) if reps else contextlib.nullcontext()
            _body_build(nc, tc, pool, psum, loop_cm,
                        tbl_in, gidx_in, cen_in, zc_in, wq_in,
                        bnred_in, bnbc_in, cb_in, out_ext,
                        rc_list, rcg_of_l, rs_v, re_v, mps, ablate)
    nc.compile()
    return nc


def _body_build(nc, tc, pool, psum, loop_cm,
                tbl_in, gidx_in, cen_in, zc_in, wq_in,
                bnred_in, bnbc_in, cb_in, out_ext,
                rc_list, rcg_of_l, rs_v, re_v, mps, ablate=()):
    pure = "pure" in ablate
    tf = sum(4 * m for m in mps[:7]) + sum(mps[7:])
    ci_main = 64 * max(mps)      # tile sizing upper bound per chunk
    with loop_cm:
            # tbl/gidx double-buffered so the next rep's input loads overlap
            # this rep's tail compute instead of serializing at the loop edge
            tbl = pool.tile([P, 2 * N], F32, tag="tbl", bufs=2)
            gidx = pool.tile([P, tf], I16, tag="gidx", bufs=2)
            cen = pool.tile([P, 2 * NSH], F32)
            zc = pool.tile([P, 4 * tf], F16)
            wq = pool.tile([P, 8], F32)
            bnred = pool.tile([P, 8], F32)
            bnbc = pool.tile([8, P], F32)
            cb = pool.tile([P, 32], F32)
            # split loads across the two HWDGE queues (SP + ACT) to overlap;
            # tbl (2MB) gates the first gather, so it's split by partition
            # halves across BOTH queues ahead of everything else
            nc.sync.dma_start(out=tbl[0:64, :], in_=tbl_in[0:64, :])
            nc.scalar.dma_start(out=tbl[64:128, :], in_=tbl_in[64:128, :])
            for t, src in [(gidx, gidx_in), (wq, wq_in),
                           (bnred, bnred_in)]:
                nc.sync.dma_start(out=t[:], in_=src[:])
            for t, src in [(zc, zc_in), (cen, cen_in),
                           (bnbc, bnbc_in), (cb, cb_in)]:
                nc.scalar.dma_start(out=t[:], in_=src[:])

            sym = pool.tile([P, OUTF], F32)
            if "pm" in ablate:
                nc.vector.memset(sym[:], 1.0)
            Rt = pool.tile([P, tf], F32)
            cen_pitch = cen[:].ap[0][0]

            # 7 full 4-ns chunks + the last chunk split into four 1-ns
            # quarters (host packs its indices (beta, nb, m)) so the
            # second-to-last stage hides under the final quarter-gathers and
            # only a tiny stage trails the last gather. Per-chunk widths
            # mps[c]; gidx column offset == Rt column offset (both ci/16
            # prefixes).
            chunks = []
            off = 0
            for k in range(7):
                chunks.append((off, 64 * mps[k], 4 * k, mps[k], 4))
                off += 4 * mps[k]
            for q in range(4):
                chunks.append((off, 16 * mps[7 + q], 28 + q, mps[7 + q], 1))
                off += mps[7 + q]
            prev_stage = None
            for colo, ci, ns0, mp, jext in chunks:
                gch = pool.tile([P, ci_main], F32, tag="gch", bufs=GCH_BUFS)
                if "gather" not in ablate:
                    nc.gpsimd.ap_gather(
                        out_ap=gch[:, 0:ci], in_ap=tbl[:],
                        idxs_ap=gidx[:, colo:colo + ci // 16],
                        channels=P, num_elems=2 * N, d=1, num_idxs=ci)
                else:
                    nc.vector.memset(gch[:], 1.0)
                if pure:
                    continue

                # the previous chunk's rsf/BN stage runs while this gather is
                # in flight; emitted BEFORE this chunk's DVE ops so the
                # in-order DVE queue doesn't head-of-line block on gch
                if prev_stage is not None and "mm" not in ablate \
                        and "quarter" not in ablate:
                    _quarter(nc, pool, psum, Rt, zc, sym, bnred, bnbc, cb,
                             out_ext, *prev_stage,
                             rc_list=rc_list, rcg_of_l=rcg_of_l, rs_v=rs_v,
                             re_v=re_v, ablate=ablate)
                prev_stage = (colo, ns0, jext, mp)

                # ---- dx^2 = (gathered - centers)^2 in ONE fused DVE pass
                # (SQDIFF custom op; in1 rank<=3 forces one call per (beta, j))
                half = ci // 2
                dxt = pool.tile([P, ci_main], F32, tag="dxt", bufs=2)
                if "prod" in ablate:
                    nc.vector.memset(dxt[:], 1.0)
                for beta in range(2):
                    if "prod" in ablate:
                        break
                    for j in range(jext):
                        cen_ap = bass.AP(
                            cen.tensor,
                            cen[:].offset + 256 * beta + ns0 + j,
                            [[cen_pitch, P], [32, 8], [0, mp]])
                        c0 = half * beta + mp * j
                        gp = [[gch[:].ap[0][0], P], [mp * jext, 8], [1, mp]]
                        nc.vector._custom_dve(
                            SQDIFF,
                            out=bass.AP(dxt.tensor, dxt[:].offset + c0, gp),
                            in0=bass.AP(gch.tensor, gch[:].offset + c0, gp),
                            in1=cen_ap)

                # ---- R^2 via PE, then R = exp(0.5 ln R^2) (stays in the
                # ln/exp ACT set; a Sqrt would force a table reload)
                if "mm" in ablate:
                    continue
                rsp = pool.tile([8, ci_main], F32, tag="rsp", bufs=1)
                for s in range(0, ci, 512):
                    w = min(512, ci - s)
                    ps = psum.tile([8, 512], F32, tag="pchunk", bufs=2)
                    nc.tensor.matmul(out=ps[:, 0:w], lhsT=wq[:],
                                     rhs=dxt[:, s:s + w],
                                     start=True, stop=True)
                    hs = slice(s, s + w)
                    nc.scalar.activation(out=rsp[0:8, hs], in_=ps[:, 0:w],
                                         func=AF.Ln)
                    nc.scalar.activation(out=rsp[0:8, hs], in_=rsp[0:8, hs],
                                         func=AF.Exp, scale=0.5)
                # SBUF->SBUF compaction [8, (p f)] -> [(g p), f]
                nc.sync.dma_start(
                    out=Rt[:, colo:colo + ci // 16],
                    in_=rsp[0:8, 0:ci].rearrange("g (p f) -> g p f", p=16))

            if not pure and "mm" not in ablate and "quarter" not in ablate:
                _quarter(nc, pool, psum, Rt, zc, sym, bnred, bnbc, cb,
                         out_ext, *prev_stage,
                         rc_list=rc_list, rcg_of_l=rcg_of_l, rs_v=rs_v,
                         re_v=re_v, ablate=ablate)


def _quarter(nc, pool, psum, Rt, zc, sym, bnred, bnbc, cb, out_ext,
             c0, ns0, nsc, mp, rc_list, rcg_of_l, rs_v, re_v, ablate=()):
    """rsf + masked reduce + BN for R columns [c0, c0 + mp*nsc), covering
    ns positions [ns0, ns0 + nsc).

    sym is the transposed accumulator [(stage, l, a, ns) blocks]; the final
    BN multiply writes through a strided AP to restore (ns, a, l) order.
    """
    W = mp * nsc
    fsl = slice(c0, c0 + W)

    c1s = []
    for gi, rcval in enumerate(rc_list):
        ur = pool.tile([P, 512], F32, tag="ur", bufs=1)
        nc.scalar.activation(out=ur[:, 0:W], in_=Rt[:, fsl], func=AF.Relu,
                             scale=-PI / rcval, bias=cb[:, 0:1])
        # cos(ur) via degree-5 polynomial in y = ur^2 (max err 2.4e-6 on
        # [0, pi]); Square/Relu live in every ACT function set, so unlike
        # Sin this costs no 1.3us table reload per use. The Horner chain
        # runs as TWO fused DVE ops (POLY3 head + POLY3B tail) instead of
        # nine stock ops.
        yy = pool.tile([P, 512], F32, tag="yy", bufs=1)
        nc.scalar.activation(out=yy[:, 0:W], in_=ur[:, 0:W], func=AF.Square)
        t = pool.tile([P, 512], F32, tag="ct", bufs=1)
        nc.vector._custom_dve(POLY3, out=t[:, 0:W], in0=yy[:, 0:W],
                              s0=COS_B[5], s1=COS_B[4], imm2=COS_B[3])
        c1 = pool.tile([P, 512], F16, tag=f"c1_{gi}")
        nc.vector._custom_dve(POLY3B, out=c1[:, 0:W], in0=t[:, 0:W],
                              in1=yy[:, 0:W], s0=COS_B[2], s1=COS_B[1],
                              imm2=COS_B[0])
        c1s.append(c1)

    # 4 type masks are host-precomputed into zc ([P, (a, tf)], plane
    # stride tf): drops 4 is_equal DVE ops per stage (DVE SBUF traffic
    # contends ~1:1 with the gather; the extra input DMA does not)
    tf4 = zc[:].ap[-1][1] // 4

    # all 12 u's and kp's in wide tiles with no ring reuse: every per-l DVE
    # chain is dependency-ready the moment Rt lands, so the tile scheduler's
    # optimistic gather timing (v1 cost model has no GPSIMD efficiency) can
    # interleave next-chunk ops into the engine order without stalling this
    # stage behind the 93us gather
    u12 = pool.tile([P, 12 * 256], F16, tag="u12", bufs=1)
    kp12 = pool.tile([P, 12 * 256], F16, tag="kp12", bufs=1)
    for l in range(L):
        nc.scalar.activation(out=u12[:, 256 * l:256 * l + W], in_=Rt[:, fsl],
                             func=AF.Square, scale=1.0,
                             bias=cb[:, 16 + l:17 + l])
    for l in range(L):
        nc.scalar.activation(out=kp12[:, 256 * l:256 * l + W],
                             in_=u12[:, 256 * l:256 * l + W], func=AF.Exp,
                             scale=-re_v[l], bias=cb[:, 3:4])
    for l in range(L):
        rsf = pool.tile([P, 512], F16, tag="rsf", bufs=2)
        nc.vector.scalar_tensor_tensor(
            out=rsf[:, 0:W], in0=c1s[rcg_of_l[l]][:, 0:W], scalar=1.0,
            in1=kp12[:, 256 * l:256 * l + W], op0=ALU.subtract,
            op1=ALU.mult)  # -K'*FCx2
        if "pm" in ablate:
            continue
        # one multiply for all 4 type masks: rsf broadcast over the a axis
        pm4 = pool.tile([P, 4 * 512], F16, tag="pm4", bufs=1)
        rsf_b = bass.AP(rsf.tensor, rsf[:].offset,
                        [[rsf[:].ap[0][0], P], [0, 4], [1, W]])
        pm4_w = bass.AP(pm4.tensor, pm4[:].offset,
                        [[pm4[:].ap[0][0], P], [512, 4], [1, W]])
        nc.vector.tensor_tensor(out=pm4_w, in0=rsf_b, in1=bass.AP(
            zc.tensor, zc[:].offset + c0,
            [[zc[:].ap[0][0], P], [tf4, 4], [1, W]]), op=ALU.mult)
        # one segmented reduce -> contiguous [128, (a, ns)] block of sym
        base = 48 * ns0 + l * 4 * nsc
        pm4_r = bass.AP(pm4.tensor, pm4[:].offset,
                        [[pm4[:].ap[0][0], P], [512, 4], [mp, nsc], [1, mp]])
        nc.vector.tensor_reduce(
            out=sym[:, base:base + 4 * nsc], in_=pm4_r,
            axis=mybir.AxisListType.X, op=ALU.add)

    # ---- batch-norm for this stage's 48*nsc sym cols [(l, a, ns) layout]
    CW = 48 * nsc
    cf = slice(48 * ns0, 48 * ns0 + CW)
    if "bn" in ablate:
        if "pm" not in ablate:
            nc.sync.dma_start(out=out_ext[:, cf], in_=sym[:, cf])
        return
    ssq = pool.tile([P, 384], F32, tag="ssq", bufs=1)
    # squares on ACT (free engine) -- DVE SBUF traffic contends ~1:1 with
    # the Q7 gather's random reads, ACT traffic does not
    nc.scalar.activation(out=ssq[:, 0:CW], in_=sym[:, cf], func=AF.Square)
    pm1 = psum.tile([8, 384], F32, tag="pbn0")
    nc.tensor.matmul(out=pm1[:, 0:CW], lhsT=bnred[:], rhs=sym[:, cf],
                     start=True, stop=True)
    pm2 = psum.tile([8, 384], F32, tag="pbn1")
    nc.tensor.matmul(out=pm2[:, 0:CW], lhsT=bnred[:], rhs=ssq[:, 0:CW],
                     start=True, stop=True)
    msb = pool.tile([8, 384], F32, tag="msb", bufs=1)
    nc.vector.tensor_copy(out=msb[0:8, 0:CW], in_=pm1[:, 0:CW])
    m2 = pool.tile([8, 384], F32, tag="m2", bufs=1)
    nc.scalar.activation(out=m2[0:8, 0:CW], in_=msb[0:8, 0:CW],
                         func=AF.Square)
    vsb = pool.tile([8, 384], F32, tag="vsb", bufs=1)
    nc.vector.tensor_tensor(out=vsb[0:8, 0:CW], in0=pm2[:, 0:CW],
                            in1=m2[0:8, 0:CW], op=ALU.subtract)
    # 1/sqrt(v + eps) = exp(-0.5 ln(v + eps)): stays in the ln/exp ACT set
    # and drops the DVE reciprocal
    ssb = pool.tile([8, 384], F32, tag="ssb", bufs=1)
    nc.scalar.activation(out=ssb[0:8, 0:CW], in_=vsb[0:8, 0:CW], func=AF.Ln,
                         bias=cb[0:8, 2:3])
    rsb = pool.tile([8, 384], F32, tag="rsb", bufs=1)
    nc.scalar.activation(out=rsb[0:8, 0:CW], in_=ssb[0:8, 0:CW], func=AF.Exp,
                         scale=-0.5)
    pbm = psum.tile([P, 384], F32, tag="pbn2")
    nc.tensor.matmul(out=pbm[:, 0:CW], lhsT=bnbc[:], rhs=msb[0:8, 0:CW],
                     start=True, stop=True)
    pbr = psum.tile([P, 384], F32, tag="pbn3")
    nc.tensor.matmul(out=pbr[:, 0:CW], lhsT=bnbc[:], rhs=rsb[0:8, 0:CW],
                     start=True, stop=True)
    dsb = pool.tile([P, 384], F32, tag="dsb", bufs=1)
    nc.vector.tensor_tensor(out=dsb[:, 0:CW], in0=pbm[:, 0:CW], in1=sym[:, cf],
                            op=ALU.subtract)
    # final multiply writes transposed: (l, a, ns) walk -> col ns*48 + a*12 + l
    osb = pool.tile([P, 384], F32, tag="osb", bufs=2)
    dsb_v = dsb[:, 0:CW].rearrange("p (l a s) -> p l a s", l=12, a=4)
    pbr_v = pbr[:, 0:CW].rearrange("p (l a s) -> p l a s", l=12, a=4)
    osb_w = bass.AP(osb.tensor, osb[:].offset,
                    [[osb[:].ap[0][0], P], [1, 12], [12, 4], [48, nsc]])
    nc.vector.tensor_tensor(out=osb_w, in0=dsb_v, in1=pbr_v, op=ALU.mult)
    nc.sync.dma_start(out=out_ext[:, cf], in_=osb[:, 0:CW])


# ---------------------------------------------------------------- host side

def make_cbias(rs_v, re_v):
    cb = np.zeros((P, 32), np.float32)
    cb[:, 0] = PI
    cb[:, 1] = 0.5 * PI
    cb[:, 2] = BN_EPS
    cb[:, 3] = math.log(0.5)
    for l in range(L):
        cb[:, 16 + l] = -float(rs_v[l])
    return cb


def compute_plan(Nbrs_Z):
    """Host plan for the Z-filter + atom sort.

    Only slots with Z in ATOM_TYPES contribute; each core's atoms are
    sorted by worst-case matching count (over its 16 batches) so that
    early chunks can use narrower per-chunk slot widths. All 8 cores run
    one compiled graph, so the widths are the cross-core envelope.

    Returns (mps, orders): mps = 11 per-chunk widths (7 main + 4 quarter
    chunks), orders[r] = per-core atom permutation (rank -> local atom).
    """
    cnt = np.isin(np.asarray(Nbrs_Z), ATOM_TYPES).sum(-1)     # [B, N]
    orders, blocks = [], []
    for r in range(8):
        c = cnt[:, NSH * r:NSH * (r + 1)]
        order = np.argsort(c.max(0), kind="stable")
        ck = c[:, order]
        blocks.append([ck[:, 32 * k:32 * k + 32].max() for k in range(7)] +
                      [ck[:, 224 + 8 * q:232 + 8 * q].max() for q in range(4)])
        orders.append(order)
    env = np.asarray(blocks).max(0)
    # even widths (f16 2x-mode wants packed pairs); otherwise exact
    mps = [int(min(M, max(8, -(-int(v) // 2) * 2))) for v in env]
    return mps, orders


def _pos2atom(order):
    """Atom at layout position 32*nb + ns is the one with sorted rank
    8*ns + nb (so chunk k's 4-ns block covers ranks [32k, 32k+32))."""
    p2a = np.empty(NSH, np.int64)
    for nb in range(8):
        for ns in range(32):
            p2a[32 * nb + ns] = order[8 * ns + nb]
    return p2a


def prep_core_inputs(X, Nbrs, Nbrs_Z, r, const_cache={}):
    """Build core r's input map (numpy layout prep only)."""
    if "plan" not in const_cache:
        const_cache["plan"] = compute_plan(Nbrs_Z)
    mps, orders = const_cache["plan"]
    p2a = _pos2atom(orders[r])
    n0 = NSH * r
    Xt = np.ascontiguousarray(X.transpose(2, 0, 1))          # [3, B, N]
    if "tbl" not in const_cache:
        tbl = np.zeros((8, 16, 2, N), np.float32)
        tbl[:, 0:3, :, :] = Xt.reshape(3, 8, 2, N).transpose(1, 0, 2, 3)
        const_cache["tbl"] = tbl.reshape(P, 2 * N)

        wq = np.zeros((P, 8), np.float32)
        for g in range(8):
            wq[16 * g + 0:16 * g + 3, g] = 1.0
        bnred = np.zeros((P, 8), np.float32)
        bnbc = np.zeros((8, P), np.float32)
        for p in range(P):
            bnred[p, p % 8] = 1.0 / 16.0
            bnbc[p % 8, p] = 1.0
        const_cache["wq"] = wq
        const_cache["bnred"] = bnred
        const_cache["bnbc"] = bnbc
        const_cache["cbias"] = None  # filled by caller

    # centers in the permuted atom order (positions, not original atoms)
    cen = np.zeros((8, 16, 2, NSH), np.float32)
    cen[:, 0:3, :, :] = (Xt[:, :, n0 + p2a]
                         .reshape(3, 8, 2, NSH).transpose(1, 0, 2, 3))
    cen = cen.reshape(P, 2 * NSH)

    # Z-filter: only slots with Z in ATOM_TYPES contribute (the masks zero
    # everything else), so keep the matching slots first and truncate each
    # (b,n) list per-chunk. Truncated-away slots are all non-matching; kept
    # non-matching slots are harmless fillers (their masks are 0).
    mpmax = max(mps)
    nbr_sh = Nbrs[:, n0:n0 + NSH, :][:, p2a, :]
    z_sh = Nbrs_Z[:, n0:n0 + NSH, :][:, p2a, :]
    fo = np.argsort(~np.isin(z_sh, ATOM_TYPES), axis=-1,
                    kind="stable")[:, :, :mpmax]
    nbr_sh = np.take_along_axis(nbr_sh, fo, axis=-1)
    z_sh = np.take_along_axis(z_sh, fo, axis=-1)

    tf = sum(4 * m for m in mps[:7]) + sum(mps[7:])
    nbr6 = nbr_sh.reshape(8, 2, 8, 8, 4, mpmax)               # [g, beta, nb, k, j, m]
    lg6 = nbr6 + (np.arange(2, dtype=nbr6.dtype)
                  .reshape(1, 2, 1, 1, 1, 1) * N)
    z6 = z_sh.reshape(8, 2, 8, 8, 4, mpmax)
    parts, zparts = [], []
    for k in range(7):
        blk = lg6[:, :, :, k, :, :mps[k]]                     # [g, b, nb, j, m]
        parts.append(blk.reshape(8, -1))
        zparts.append(z6[:, :, :, k, :, :mps[k]].reshape(8, 2, 8, -1))
    for q in range(4):
        blk = lg6[:, :, :, 7, q, :mps[7 + q]]                 # [g, b, nb, m]
        parts.append(blk.reshape(8, -1))
        zparts.append(z6[:, :, :, 7, q, :mps[7 + q]].reshape(8, 2, 8, -1))
    lg = np.concatenate(parts, axis=1)                        # [8, 16*tf]
    gidx = (lg.reshape(8, tf, 16).transpose(0, 2, 1)
            .reshape(P, tf).astype(np.int16))
    zraw = np.concatenate(zparts, axis=3).reshape(P, tf)
    zc = np.concatenate([(zraw == t) for t in ATOM_TYPES],
                        axis=1).astype(np.float16)

    return {"tbl": const_cache["tbl"], "gidx": gidx, "cen": cen, "zc": zc,
            "wq": const_cache["wq"], "bnred": const_cache["bnred"],
            "bnbc": const_cache["bnbc"], "cbias": const_cache["cbias"]}


def assemble_output(results, orders=None):
    full = np.empty((8, 2, N, NFEAT), np.float32)             # [g, beta, n, f]
    for r in range(8):
        o = np.asarray(results[r]["out"]).reshape(8, 2, NSH, NFEAT)
        n0 = NSH * r
        if orders is None:
            full[:, :, n0:n0 + NSH, :] = o
        else:
            # position 32*nb + ns holds the atom with sorted rank 8*ns + nb
            full[:, :, n0 + _pos2atom(orders[r]), :] = o
    return full.reshape(B, N, NFEAT)


_cache = {}


def kernel(X, Nbrs, Nbrs_Z, rc, rs, re):
    from concourse.bass_utils import run_bass_kernel_spmd
    Nbrs_Z = np.asarray(Nbrs_Z)
    plan = compute_plan(Nbrs_Z)
    mps = plan[0]
    key = (tuple(np.asarray(rc).ravel().tolist()),
           tuple(np.asarray(rs).ravel().tolist()),
           tuple(np.asarray(re).ravel().tolist()), tuple(mps))
    if key not in _cache:
        _cache[key] = build_nc(np.asarray(rc).ravel(), np.asarray(rs).ravel(),
                               np.asarray(re).ravel(), mps=mps)
    nc = _cache[key]
    X = np.asarray(X, np.float32)
    Nbrs = np.asarray(Nbrs)
    cc = {"plan": plan}
    in_maps = [prep_core_inputs(X, Nbrs, Nbrs_Z, r, cc) for r in range(8)]
    cbias = make_cbias(np.asarray(rs).ravel(), np.asarray(re).ravel())
    for im in in_maps:
        im["cbias"] = cbias
    res = run_bass_kernel_spmd(nc, in_maps, core_ids=list(range(8)))
    return assemble_output(res.results, plan[1])



# revision 32
# speedup vs baseline: 1.2016x; 1.2016x over previous
"""AtomicConvolution Trainium2 kernel (8 NeuronCores, SPMD, no collectives).

Sharding: N-shard. Core r handles atoms [256r, 256r+256) for ALL 16 batches.
The X coordinate table (tiny) is replicated per core, so the neighbor gather
is core-local and the batch-norm moments over axis 0 (batch) are core-local
too (each core holds all 16 batches for its atoms). No cross-core traffic.

The GPSIMD ap_gather is the bottleneck at ~23-32ns/group-index (device-state
dependent), and measurement shows the rate is pinned by the SBUF random-read
path itself: the native Pool-engine IndirectCopy, f16 tables, small tables,
d>1, and any instruction chunking all land at the same per-index cost, and
the DMA-engine gather (InstDMAGatherAnt, mlp library - it does work on this
firmware, the old crash note is stale) is WORSE: its Q7 descriptor
generation costs ~8ns/idx on the same sequencer that runs ap_gather, so
hybrids lose. The only real lever is gathering FEWER slots:

  - Z-FILTER (the big win): Nbrs_Z is drawn from [0,10) but only types
    {1,6,7,8} match a mask plane, so ~60% of slots are multiplied by zero
    in every output. The host keeps matching slots first (stable argsort of
    ~isin) and truncates each (b,n) list; kept non-matching slots are
    harmless fillers. Exactly correct, not an approximation.
  - ATOM SORT + PER-CHUNK WIDTHS: each core's atoms are sorted by their
    worst-case (max over 16 batches) matching count and laid out so chunk k
    covers sorted ranks [32k, 32k+32); chunk widths mps[k] are the
    cross-core envelope of the per-chunk maxima (one compiled graph runs on
    all 8 cores), rounded up to 2 (even widths keep f16 2x DVE modes). For
    the fixed harness dataset this gives [30,32,32,34,34,34,36 |
    36,38,38,48] = ~30% fewer gather indices than uniform 48 and ~47% fewer
    than the unfiltered 64. The permutation is undone on host in
    assemble_output.

Per-core pipeline per chunk (stage s hides under gather s+1):
  - gather table tbl[16g+c, beta*2048 + j] = plane c of X[2g+beta, j]
    (partition-group g owns batches {2g, 2g+1}; c=0,1,2 -> x,y,z; c=3 -> 0)
  - ap_gather (d=1, 7 main chunks of 64*mp idx/group + the last chunk as
    four 16*mp quarters so only a tiny stage trails the final gather;
    2-deep output ring) pulls all 16 channels per index -> x,y,z per read
  - dx^2 = (gathered - centers)^2 in ONE fused custom-DVE op (SQDIFF,
    registered at import into dve_ops.OPS; one call per (beta, j) since
    custom-DVE APs are rank<=3); R^2 = PE ones-block matmul reducing the 4
    channels; R = exp(0.5 ln R^2) on ACT (stays in the ln/exp table set);
    one SBUF->SBUF DMA compacts R to [128, tf] with partition p = 8b + nb,
    free = per-chunk (j, m) blocks at ci/16-prefix offsets (gidx column
    offset == Rt column offset)
  - rsf_l = exp(-re(R-rs)^2) * 0.5*(cos(pi R/rc)+1) * [R<=rc]:
      u = Square(R - rs) (ACT); K' = Exp(-re*u + ln 0.5) (ACT)
      cos via degree-5 polynomial in y = Relu(pi - pi R/rc)^2 (ACT Relu +
      ACT Square + TWO fused Horner custom-DVE ops POLY3/POLY3B; exact
      cutoff, max err 2.4e-6)
      rsf = (1 - cos)*K' via one stock scalar_tensor_tensor
  - per l: ONE broadcast multiply against the packed 4-type mask + ONE
    segmented reduce into a transposed sym accumulator; the R path stays
    f32 (exp(-re(R-rs)^2) is brutally sensitive to R error) but the
    post-exp path runs in fp16 (2x DVE modes, less SBUF traffic: measured
    DVE SBUF traffic during gathers costs ~1:1 in gather slowdown, while
    ACT traffic is free - keep DVE lean, ACT can soak work)
  - BN over the 16 batches: PE stride-8 partition reductions + broadcasts;
    the final multiply writes through a strided AP to restore (ns, a*12+l)
    output order. Stages are emitted BEFORE the next chunk's DVE ops so the
    in-order DVE queue never head-of-line blocks on the gather.

End-to-end rel err ~1.7e-3 vs the 2e-2 gate (identical to the unfiltered
baseline - the filter/sort/widths change is exact). Measured on-device
(median of interleaved reps=257 pair differences; wall-clock pairing drifts
by tens of ms mid-session so only adjacent same-state comparisons count):
baseline 1.32ms -> Z-filter mp=48 + fused DVE ops: -407us/rep -> adaptive
per-chunk widths: -227us/rep more (~603-642us absolute); roundup2 widths
and a partition-split tbl load (the 2MB table DMA gates the first gather;
halves go on both HWDGE queues) shave a further few percent of indices and
head latency.
"""
import sys

if '/opt/trn_rl_repo' not in sys.path:
    sys.path.insert(0, '/opt/trn_rl_repo')

import math
import numpy as np

import concourse.bacc as bacc
import concourse.bass as bass
import concourse.mybir as mybir
from concourse import library_config
from concourse.tile import TileContext

# Steer the act-table-load pass: every ACT func this kernel uses (exp, ln,
# square, relu) lives in natural_log_exp_and_others, but the insertion pass
# maps each func to the FIRST set containing it, so ln<->exp would reload the
# table on every switch. Strip those funcs from all other sets (set order and
# ids are unchanged) so everything resolves to the one shared set and the
# single load hoists out of the loop.
import concourse.hw_specs as _hw_specs
if not getattr(_hw_specs, "_act_tbl_patched", False):
    _orig_gat = _hw_specs.get_activation_tables

    def _gat_one_set(arch):
        tabs = _orig_gat(arch)
        keep = "natural_log_exp_and_others"
        if keep not in tabs:
            return tabs
        shared = tabs[keep]
        return {name: (fs if name == keep else fs - shared)
                for name, fs in tabs.items()}

    _hw_specs.get_activation_tables = _gat_one_set
    _hw_specs._act_tbl_patched = True
    bacc.get_activation_tables = _gat_one_set

# Give the Tile scheduler realistic ap_gather timing. The stock cost model
# rates a 4096-idx gather at ~5.7us; measured hardware is ~93us. With the
# stock number the static schedule believes gathers finish instantly and
# orders the next chunk's DVE ops ahead of the previous R-stage, so the
# in-order DVE queue head-of-line blocks on the gather and ~120us of stage
# work drains serially after the last gather. 0.0366 calibrates the model
# to 22.8ns/group-idx. Must run before the first build in the process (the
# rust side caches the spec in a OnceLock).
if "APGather" not in _hw_specs.TRN2Spec.GPSIMD_IMPL_EFFICIENCY:
    for _k in ("APGather", "InstAPGather", "ISA"):
        _hw_specs.TRN2Spec.GPSIMD_IMPL_EFFICIENCY[_k] = 0.0366

F32 = mybir.dt.float32
F16 = mybir.dt.float16
BF16 = mybir.dt.bfloat16
I16 = mybir.dt.int16
AF = mybir.ActivationFunctionType
ALU = mybir.AluOpType

# ---- runtime-registered fused DVE ops (fewer passes -> less SBUF traffic
# concurrent with the Q7 gather's random reads, which measurement shows is
# nearly 1:1 additive with gather time).
import numpy as _np
import concourse.dve_ops as _dve_ops
from concourse.dve_spec import (Spec as _Spec, Src0 as _S0, Src1 as _S1,
                                C0 as _DC0, C1 as _DC1, C2 as _DC2,
                                sq as _sq, lower as _dve_lower,
                                _has_src1 as _dve_has_src1)
from concourse.dve_uop import DveOpSpec as _DveOpSpec


def _reg_dve(name, spec):
    for o in _dve_ops.OPS:
        if o.name == name:
            return o
    row = max(_dve_ops._SUB_OPCODE_FOR_NAME.values()) + 1
    assert row < 0x20
    tmp = _DveOpSpec(name=name, opcode=row, uops=_dve_lower(spec, ver="v3"),
                     rd1_en=_dve_has_src1(spec))
    op = _dve_ops.DveOp(name, spec, subdim=False,
                        uops_sha={"v3": tmp.sha("v3")})
    _dve_ops.OPS.append(op)
    _dve_ops.CUSTOM_DVE_SPECS[name] = spec
    _dve_ops._SUB_OPCODE_FOR_NAME[name] = row
    return op


# out = (in0 - in1)^2   (dx then dx*dx in one pass)
SQDIFF = _reg_dve("SQDIFF_ANT", _Spec(
    body=_sq(_S0 - _S1),
    reference=lambda in0, in1, s0, s1, imm2:
        ((in0.astype(_np.float32) - in1) ** 2).astype(_np.float32)))
# out = (s0*in0 + s1)*in0 + imm2   (Horner head of the cos polynomial)
POLY3 = _reg_dve("POLY3_ANT", _Spec(
    body=(_DC0 * _S0 + _DC1) * _S0 + _DC2,
    reference=lambda in0, in1, s0, s1, imm2:
        ((s0 * in0.astype(_np.float32) + s1) * in0 + imm2).astype(_np.float32)))
# out = ((in0*in1 + s0)*in1 + s1)*in1 + imm2   (Horner tail)
POLY3B = _reg_dve("POLY3B_ANT", _Spec(
    body=((_S0 * _S1 + _DC0) * _S1 + _DC1) * _S1 + _DC2,
    reference=lambda in0, in1, s0, s1, imm2:
        (((in0.astype(_np.float32) * in1 + s0) * in1 + s1) * in1 + imm2
         ).astype(_np.float32)))

P = 128
B, N, M, L, A = 16, 2048, 64, 12, 4
NSH = N // 8                 # atoms per core = 256
NFEAT = A * L                # 48
OUTF = 32 * NFEAT            # 1536 output cols per partition
ATOM_TYPES = (1, 6, 7, 8)
BN_EPS = 1e-3
PI = math.pi
GCH_BUFS = 2
# cos(x) on [0, pi] as a degree-5 polynomial in y = x^2 (max err 2.4e-6)
COS_B = (0.9999994437, -0.4999955817, 0.0416610328, -0.0013862747,
         2.42532e-05, -2.219e-07)


def build_nc(rc_v, rs_v, re_v, reps=None, ablate=(), mp=64, mps=None):
    """Build the per-core graph. rc/rs/re are baked in as immediates.
    reps: if set, wrap the whole body in a HW For_i loop (for benchmarking).
    ablate: subset of {"gather","prod","mm","quarter"} to skip (profiling).
    mps: per-chunk slot widths after the host's Z-filter, 11 entries (7 main
    chunks of 4 ns + 4 quarter chunks of 1 ns). Only slots whose Z matches
    one of the 4 atom types contribute to the output, so the host compacts
    each neighbor list to the matching slots (padded with non-matching
    ones, which the type masks zero out exactly). The host additionally
    sorts each core's atoms by worst-case matching count (undone at output
    assembly) so early chunks get narrower widths. mp is the uniform
    fallback when mps is None."""
    ablate = set(ablate)
    if mps is None:
        mps = [mp] * 11
    rc_v = [float(x) for x in rc_v]
    rs_v = [float(x) for x in rs_v]
    re_v = [float(x) for x in re_v]
    rc_groups = {}
    for l, v in enumerate(rc_v):
        rc_groups.setdefault(v, []).append(l)
    rc_list = list(rc_groups.keys())
    rcg_of_l = {}
    for gi, v in enumerate(rc_list):
        for l in rc_groups[v]:
            rcg_of_l[l] = gi

    tf = sum(4 * m for m in mps[:7]) + sum(mps[7:])  # compacted R free size
    nc = bacc.Bacc()
    tbl_in = nc.declare_dram_parameter("tbl", [P, 2 * N], F32, isOutput=False)
    gidx_in = nc.declare_dram_parameter("gidx", [P, tf], I16, isOutput=False)
    cen_in = nc.declare_dram_parameter("cen", [P, 2 * NSH], F32, isOutput=False)
    zc_in = nc.declare_dram_parameter("zc", [P, tf], F16, isOutput=False)
    wq_in = nc.declare_dram_parameter("wq", [P, 8], F16, isOutput=False)
    bnred_in = nc.declare_dram_parameter("bnred", [P, 8], F32, isOutput=False)
    bnbc_in = nc.declare_dram_parameter("bnbc", [8, P], F32, isOutput=False)
    cb_in = nc.declare_dram_parameter("cbias", [P, 32], F32, isOutput=False)
    out_ext = nc.declare_dram_parameter("out", [P, OUTF], F32, isOutput=True)

    import contextlib
    with TileContext(nc) as tc:
        with tc.tile_pool(name="sbuf", bufs=1) as pool, \
             tc.tile_pool(name="psum", bufs=1, space="PSUM") as psum:
            nc.gpsimd.load_library(library_config.ap_gather)
            loop_cm = tc.For_i(0, reps, 1) if reps else contextlib.nullcontext()
            _body_build(nc, tc, pool, psum, loop_cm,
                        tbl_in, gidx_in, cen_in, zc_in, wq_in,
                        bnred_in, bnbc_in, cb_in, out_ext,
                        rc_list, rcg_of_l, rs_v, re_v, mps, ablate)
    nc.compile()
    return nc


def _body_build(nc, tc, pool, psum, loop_cm,
                tbl_in, gidx_in, cen_in, zc_in, wq_in,
                bnred_in, bnbc_in, cb_in, out_ext,
                rc_list, rcg_of_l, rs_v, re_v, mps, ablate=()):
    pure = "pure" in ablate
    tf = sum(4 * m for m in mps[:7]) + sum(mps[7:])
    ci_main = 64 * max(mps)      # tile sizing upper bound per chunk
    with loop_cm:
            # tbl/gidx double-buffered so the next rep's input loads overlap
            # this rep's tail compute instead of serializing at the loop edge
            tbl = pool.tile([P, 2 * N], F32, tag="tbl", bufs=2)
            gidx = pool.tile([P, tf], I16, tag="gidx", bufs=2)
            cen = pool.tile([P, 2 * NSH], F32)
            zc = pool.tile([P, tf], F16)
            wq = pool.tile([P, 8], F16)
            bnred = pool.tile([P, 8], F32)
            bnbc = pool.tile([8, P], F32)
            cb = pool.tile([P, 32], F32)
            # split loads across the two HWDGE queues (SP + ACT) to overlap;
            # tbl (2MB) gates the first gather, so it's split by partition
            # halves across BOTH queues ahead of everything else
            nc.sync.dma_start(out=tbl[0:64, :], in_=tbl_in[0:64, :])
            nc.scalar.dma_start(out=tbl[64:128, :], in_=tbl_in[64:128, :])
            for t, src in [(gidx, gidx_in), (wq, wq_in),
                           (bnred, bnred_in)]:
                nc.sync.dma_start(out=t[:], in_=src[:])
            for t, src in [(zc, zc_in), (cen, cen_in),
                           (bnbc, bnbc_in), (cb, cb_in)]:
                nc.scalar.dma_start(out=t[:], in_=src[:])

            sym = pool.tile([P, OUTF], F32)
            if "pm" in ablate:
                nc.vector.memset(sym[:], 1.0)
            Rt = pool.tile([P, tf], F32)
            cen_pitch = cen[:].ap[0][0]

            # 7 full 4-ns chunks + the last chunk split into four 1-ns
            # quarters (host packs its indices (beta, nb, m)) so the
            # second-to-last stage hides under the final quarter-gathers and
            # only a tiny stage trails the last gather. Per-chunk widths
            # mps[c]; gidx column offset == Rt column offset (both ci/16
            # prefixes).
            chunks = []
            off = 0
            for k in range(7):
                chunks.append((off, 64 * mps[k], 4 * k, mps[k], 4))
                off += 4 * mps[k]
            for q in range(4):
                chunks.append((off, 16 * mps[7 + q], 28 + q, mps[7 + q], 1))
                off += mps[7 + q]
            prev_stage = None
            for colo, ci, ns0, mp, jext in chunks:
                gch = pool.tile([P, ci_main], F32, tag="gch", bufs=GCH_BUFS)
                if "gather" not in ablate:
                    nc.gpsimd.ap_gather(
                        out_ap=gch[:, 0:ci], in_ap=tbl[:],
                        idxs_ap=gidx[:, colo:colo + ci // 16],
                        channels=P, num_elems=2 * N, d=1, num_idxs=ci)
                else:
                    nc.vector.memset(gch[:], 1.0)
                if pure:
                    continue

                # the previous chunk's rsf/BN stage runs while this gather is
                # in flight; emitted BEFORE this chunk's DVE ops so the
                # in-order DVE queue doesn't head-of-line block on gch
                if prev_stage is not None and "mm" not in ablate \
                        and "quarter" not in ablate:
                    _quarter(nc, pool, psum, Rt, zc, sym, bnred, bnbc, cb,
                             out_ext, *prev_stage,
                             rc_list=rc_list, rcg_of_l=rcg_of_l, rs_v=rs_v,
                             re_v=re_v, ablate=ablate)
                prev_stage = (colo, ns0, jext, mp)

                # ---- dx^2 = (gathered - centers)^2 in ONE fused DVE pass
                # (SQDIFF custom op; in1 rank<=3 forces one call per (beta, j))
                half = ci // 2
                dxt = pool.tile([P, ci_main], F16, tag="dxt", bufs=2)
                if "prod" in ablate:
                    nc.vector.memset(dxt[:], 1.0)
                for beta in range(2):
                    if "prod" in ablate:
                        break
                    for j in range(jext):
                        cen_ap = bass.AP(
                            cen.tensor,
                            cen[:].offset + 256 * beta + ns0 + j,
                            [[cen_pitch, P], [32, 8], [0, mp]])
                        c0 = half * beta + mp * j
                        gp = [[gch[:].ap[0][0], P], [mp * jext, 8], [1, mp]]
                        gpo = [[dxt[:].ap[0][0], P], [mp * jext, 8], [1, mp]]
                        nc.vector._custom_dve(
                            SQDIFF,
                            out=bass.AP(dxt.tensor, dxt[:].offset + c0, gpo),
                            in0=bass.AP(gch.tensor, gch[:].offset + c0, gp),
                            in1=cen_ap)

                # ---- R^2 via PE, then R = exp(0.5 ln R^2) (stays in the
                # ln/exp ACT set; a Sqrt would force a table reload)
                if "mm" in ablate:
                    continue
                rsp = pool.tile([8, ci_main], F32, tag="rsp", bufs=1)
                for s in range(0, ci, 512):
                    w = min(512, ci - s)
                    ps = psum.tile([8, 512], F32, tag="pchunk", bufs=2)
                    nc.tensor.matmul(out=ps[:, 0:w], lhsT=wq[:],
                                     rhs=dxt[:, s:s + w],
                                     start=True, stop=True)
                    hs = slice(s, s + w)
                    nc.scalar.activation(out=rsp[0:8, hs], in_=ps[:, 0:w],
                                         func=AF.Ln)
                    nc.scalar.activation(out=rsp[0:8, hs], in_=rsp[0:8, hs],
                                         func=AF.Exp, scale=0.5)
                # SBUF->SBUF compaction [8, (p f)] -> [(g p), f]
                nc.sync.dma_start(
                    out=Rt[:, colo:colo + ci // 16],
                    in_=rsp[0:8, 0:ci].rearrange("g (p f) -> g p f", p=16))

            if not pure and "mm" not in ablate and "quarter" not in ablate:
                _quarter(nc, pool, psum, Rt, zc, sym, bnred, bnbc, cb,
                         out_ext, *prev_stage,
                         rc_list=rc_list, rcg_of_l=rcg_of_l, rs_v=rs_v,
                         re_v=re_v, ablate=ablate)


def _quarter(nc, pool, psum, Rt, zc, sym, bnred, bnbc, cb, out_ext,
             c0, ns0, nsc, mp, rc_list, rcg_of_l, rs_v, re_v, ablate=()):
    """rsf + masked reduce + BN for R columns [c0, c0 + mp*nsc), covering
    ns positions [ns0, ns0 + nsc).

    sym is the transposed accumulator [(stage, l, a, ns) blocks]; the final
    BN multiply writes through a strided AP to restore (ns, a, l) order.
    """
    W = mp * nsc
    fsl = slice(c0, c0 + W)

    c1s = []
    for gi, rcval in enumerate(rc_list):
        ur = pool.tile([P, 512], F32, tag="ur", bufs=1)
        nc.scalar.activation(out=ur[:, 0:W], in_=Rt[:, fsl], func=AF.Relu,
                             scale=-PI / rcval, bias=cb[:, 0:1])
        # cos(ur) via degree-5 polynomial in y = ur^2 (max err 2.4e-6 on
        # [0, pi]); Square/Relu live in every ACT function set, so unlike
        # Sin this costs no 1.3us table reload per use. The Horner chain
        # runs as TWO fused DVE ops (POLY3 head + POLY3B tail) instead of
        # nine stock ops.
        yy = pool.tile([P, 512], F32, tag="yy", bufs=1)
        nc.scalar.activation(out=yy[:, 0:W], in_=ur[:, 0:W], func=AF.Square)
        t = pool.tile([P, 512], F32, tag="ct", bufs=1)
        nc.vector._custom_dve(POLY3, out=t[:, 0:W], in0=yy[:, 0:W],
                              s0=COS_B[5], s1=COS_B[4], imm2=COS_B[3])
        c1 = pool.tile([P, 512], F16, tag=f"c1_{gi}")
        nc.vector._custom_dve(POLY3B, out=c1[:, 0:W], in0=t[:, 0:W],
                              in1=yy[:, 0:W], s0=COS_B[2], s1=COS_B[1],
                              imm2=COS_B[0])
        c1s.append(c1)

    # 4 type masks packed [128, (a, i)] so each l needs ONE mask multiply
    mask4 = pool.tile([P, 4 * 512], F16, tag="mask4", bufs=1)
    for a in range(A):
        nc.vector.tensor_scalar(out=mask4[:, 512 * a:512 * a + W],
                                in0=zc[:, fsl],
                                scalar1=float(ATOM_TYPES[a]), scalar2=None,
                                op0=ALU.is_equal)

    # all 12 u's and kp's in wide tiles with no ring reuse: every per-l DVE
    # chain is dependency-ready the moment Rt lands, so the tile scheduler's
    # optimistic gather timing (v1 cost model has no GPSIMD efficiency) can
    # interleave next-chunk ops into the engine order without stalling this
    # stage behind the 93us gather
    u12 = pool.tile([P, 12 * 256], F16, tag="u12", bufs=1)
    kp12 = pool.tile([P, 12 * 256], F16, tag="kp12", bufs=1)
    for l in range(L):
        nc.scalar.activation(out=u12[:, 256 * l:256 * l + W], in_=Rt[:, fsl],
                             func=AF.Square, scale=1.0,
                             bias=cb[:, 16 + l:17 + l])
    for l in range(L):
        nc.scalar.activation(out=kp12[:, 256 * l:256 * l + W],
                             in_=u12[:, 256 * l:256 * l + W], func=AF.Exp,
                             scale=-re_v[l], bias=cb[:, 3:4])
    for l in range(L):
        rsf = pool.tile([P, 512], F16, tag="rsf", bufs=2)
        nc.vector.scalar_tensor_tensor(
            out=rsf[:, 0:W], in0=c1s[rcg_of_l[l]][:, 0:W], scalar=1.0,
            in1=kp12[:, 256 * l:256 * l + W], op0=ALU.subtract,
            op1=ALU.mult)  # -K'*FCx2
        if "pm" in ablate:
            continue
        # one multiply for all 4 type masks: rsf broadcast over the a axis
        pm4 = pool.tile([P, 4 * 512], F16, tag="pm4", bufs=1)
        rsf_b = bass.AP(rsf.tensor, rsf[:].offset,
                        [[rsf[:].ap[0][0], P], [0, 4], [1, W]])
        pm4_w = bass.AP(pm4.tensor, pm4[:].offset,
                        [[pm4[:].ap[0][0], P], [512, 4], [1, W]])
        nc.vector.tensor_tensor(out=pm4_w, in0=rsf_b, in1=bass.AP(
            mask4.tensor, mask4[:].offset,
            [[mask4[:].ap[0][0], P], [512, 4], [1, W]]), op=ALU.mult)
        # one segmented reduce -> contiguous [128, (a, ns)] block of sym
        base = 48 * ns0 + l * 4 * nsc
        pm4_r = bass.AP(pm4.tensor, pm4[:].offset,
                        [[pm4[:].ap[0][0], P], [512, 4], [mp, nsc], [1, mp]])
        nc.vector.tensor_reduce(
            out=sym[:, base:base + 4 * nsc], in_=pm4_r,
            axis=mybir.AxisListType.X, op=ALU.add)

    # ---- batch-norm for this stage's 48*nsc sym cols [(l, a, ns) layout]
    CW = 48 * nsc
    cf = slice(48 * ns0, 48 * ns0 + CW)
    if "bn" in ablate:
        if "pm" not in ablate:
            nc.sync.dma_start(out=out_ext[:, cf], in_=sym[:, cf])
        return
    ssq = pool.tile([P, 384], F32, tag="ssq", bufs=1)
    # squares on ACT (free engine) -- DVE SBUF traffic contends ~1:1 with
    # the Q7 gather's random reads, ACT traffic does not
    nc.scalar.activation(out=ssq[:, 0:CW], in_=sym[:, cf], func=AF.Square)
    pm1 = psum.tile([8, 384], F32, tag="pbn0")
    nc.tensor.matmul(out=pm1[:, 0:CW], lhsT=bnred[:], rhs=sym[:, cf],
                     start=True, stop=True)
    pm2 = psum.tile([8, 384], F32, tag="pbn1")
    nc.tensor.matmul(out=pm2[:, 0:CW], lhsT=bnred[:], rhs=ssq[:, 0:CW],
                     start=True, stop=True)
    msb = pool.tile([8, 384], F32, tag="msb", bufs=1)
    nc.vector.tensor_copy(out=msb[0:8, 0:CW], in_=pm1[:, 0:CW])
    m2 = pool.tile([8, 384], F32, tag="m2", bufs=1)
    nc.scalar.activation(out=m2[0:8, 0:CW], in_=msb[0:8, 0:CW],
                         func=AF.Square)
    vsb = pool.tile([8, 384], F32, tag="vsb", bufs=1)
    nc.vector.tensor_tensor(out=vsb[0:8, 0:CW], in0=pm2[:, 0:CW],
                            in1=m2[0:8, 0:CW], op=ALU.subtract)
    # 1/sqrt(v + eps) = exp(-0.5 ln(v + eps)): stays in the ln/exp ACT set
    # and drops the DVE reciprocal
    ssb = pool.tile([8, 384], F32, tag="ssb", bufs=1)
    nc.scalar.activation(out=ssb[0:8, 0:CW], in_=vsb[0:8, 0:CW], func=AF.Ln,
                         bias=cb[0:8, 2:3])
    rsb = pool.tile([8, 384], F32, tag="rsb", bufs=1)
    nc.scalar.activation(out=rsb[0:8, 0:CW], in_=ssb[0:8, 0:CW], func=AF.Exp,
                         scale=-0.5)
    pbm = psum.tile([P, 384], F32, tag="pbn2")
    nc.tensor.matmul(out=pbm[:, 0:CW], lhsT=bnbc[:], rhs=msb[0:8, 0:CW],
                     start=True, stop=True)
    pbr = psum.tile([P, 384], F32, tag="pbn3")
    nc.tensor.matmul(out=pbr[:, 0:CW], lhsT=bnbc[:], rhs=rsb[0:8, 0:CW],
                     start=True, stop=True)
    dsb = pool.tile([P, 384], F32, tag="dsb", bufs=1)
    nc.vector.tensor_tensor(out=dsb[:, 0:CW], in0=pbm[:, 0:CW], in1=sym[:, cf],
                            op=ALU.subtract)
    # final multiply writes transposed: (l, a, ns) walk -> col ns*48 + a*12 + l
    osb = pool.tile([P, 384], F32, tag="osb", bufs=2)
    dsb_v = dsb[:, 0:CW].rearrange("p (l a s) -> p l a s", l=12, a=4)
    pbr_v = pbr[:, 0:CW].rearrange("p (l a s) -> p l a s", l=12, a=4)
    osb_w = bass.AP(osb.tensor, osb[:].offset,
                    [[osb[:].ap[0][0], P], [1, 12], [12, 4], [48, nsc]])
    nc.vector.tensor_tensor(out=osb_w, in0=dsb_v, in1=pbr_v, op=ALU.mult)
    nc.sync.dma_start(out=out_ext[:, cf], in_=osb[:, 0:CW])


# ---------------------------------------------------------------- host side

def make_cbias(rs_v, re_v):
    cb = np.zeros((P, 32), np.float32)
    cb[:, 0] = PI
    cb[:, 1] = 0.5 * PI
    cb[:, 2] = BN_EPS
    cb[:, 3] = math.log(0.5)
    for l in range(L):
        cb[:, 16 + l] = -float(rs_v[l])
    return cb


def compute_plan(Nbrs_Z):
    """Host plan for the Z-filter + atom sort.

    Only slots with Z in ATOM_TYPES contribute; each core's atoms are
    sorted by worst-case matching count (over its 16 batches) so that
    early chunks can use narrower per-chunk slot widths. All 8 cores run
    one compiled graph, so the widths are the cross-core envelope.

    Returns (mps, orders): mps = 11 per-chunk widths (7 main + 4 quarter
    chunks), orders[r] = per-core atom permutation (rank -> local atom).
    """
    cnt = np.isin(np.asarray(Nbrs_Z), ATOM_TYPES).sum(-1)     # [B, N]
    orders, blocks = [], []
    for r in range(8):
        c = cnt[:, NSH * r:NSH * (r + 1)]
        order = np.argsort(c.max(0), kind="stable")
        ck = c[:, order]
        blocks.append([ck[:, 32 * k:32 * k + 32].max() for k in range(7)] +
                      [ck[:, 224 + 8 * q:232 + 8 * q].max() for q in range(4)])
        orders.append(order)
    env = np.asarray(blocks).max(0)
    # even widths (f16 2x-mode wants packed pairs); otherwise exact
    mps = [int(min(M, max(8, -(-int(v) // 2) * 2))) for v in env]
    return mps, orders


def _pos2atom(order):
    """Atom at layout position 32*nb + ns is the one with sorted rank
    8*ns + nb (so chunk k's 4-ns block covers ranks [32k, 32k+32))."""
    p2a = np.empty(NSH, np.int64)
    for nb in range(8):
        for ns in range(32):
            p2a[32 * nb + ns] = order[8 * ns + nb]
    return p2a


def prep_core_inputs(X, Nbrs, Nbrs_Z, r, const_cache={}):
    """Build core r's input map (numpy layout prep only)."""
    if "plan" not in const_cache:
        const_cache["plan"] = compute_plan(Nbrs_Z)
    mps, orders = const_cache["plan"]
    p2a = _pos2atom(orders[r])
    n0 = NSH * r
    Xt = np.ascontiguousarray(X.transpose(2, 0, 1))          # [3, B, N]
    if "tbl" not in const_cache:
        tbl = np.zeros((8, 16, 2, N), np.float32)
        tbl[:, 0:3, :, :] = Xt.reshape(3, 8, 2, N).transpose(1, 0, 2, 3)
        const_cache["tbl"] = tbl.reshape(P, 2 * N)

        wq = np.zeros((P, 8), np.float32)
        for g in range(8):
            wq[16 * g + 0:16 * g + 3, g] = 1.0
        bnred = np.zeros((P, 8), np.float32)
        bnbc = np.zeros((8, P), np.float32)
        for p in range(P):
            bnred[p, p % 8] = 1.0 / 16.0
            bnbc[p % 8, p] = 1.0
        const_cache["wq"] = wq.astype(np.float16)
        const_cache["bnred"] = bnred
        const_cache["bnbc"] = bnbc
        const_cache["cbias"] = None  # filled by caller

    # centers in the permuted atom order (positions, not original atoms)
    cen = np.zeros((8, 16, 2, NSH), np.float32)
    cen[:, 0:3, :, :] = (Xt[:, :, n0 + p2a]
                         .reshape(3, 8, 2, NSH).transpose(1, 0, 2, 3))
    cen = cen.reshape(P, 2 * NSH)

    # Z-filter: only slots with Z in ATOM_TYPES contribute (the masks zero
    # everything else), so keep the matching slots first and truncate each
    # (b,n) list per-chunk. Truncated-away slots are all non-matching; kept
    # non-matching slots are harmless fillers (their masks are 0).
    mpmax = max(mps)
    nbr_sh = Nbrs[:, n0:n0 + NSH, :][:, p2a, :]
    z_sh = Nbrs_Z[:, n0:n0 + NSH, :][:, p2a, :]
    fo = np.argsort(~np.isin(z_sh, ATOM_TYPES), axis=-1,
                    kind="stable")[:, :, :mpmax]
    nbr_sh = np.take_along_axis(nbr_sh, fo, axis=-1)
    z_sh = np.take_along_axis(z_sh, fo, axis=-1)

    tf = sum(4 * m for m in mps[:7]) + sum(mps[7:])
    nbr6 = nbr_sh.reshape(8, 2, 8, 8, 4, mpmax)               # [g, beta, nb, k, j, m]
    lg6 = nbr6 + (np.arange(2, dtype=nbr6.dtype)
                  .reshape(1, 2, 1, 1, 1, 1) * N)
    z6 = z_sh.reshape(8, 2, 8, 8, 4, mpmax)
    parts, zparts = [], []
    for k in range(7):
        blk = lg6[:, :, :, k, :, :mps[k]]                     # [g, b, nb, j, m]
        parts.append(blk.reshape(8, -1))
        zparts.append(z6[:, :, :, k, :, :mps[k]].reshape(8, 2, 8, -1))
    for q in range(4):
        blk = lg6[:, :, :, 7, q, :mps[7 + q]]                 # [g, b, nb, m]
        parts.append(blk.reshape(8, -1))
        zparts.append(z6[:, :, :, 7, q, :mps[7 + q]].reshape(8, 2, 8, -1))
    lg = np.concatenate(parts, axis=1)                        # [8, 16*tf]
    gidx = (lg.reshape(8, tf, 16).transpose(0, 2, 1)
            .reshape(P, tf).astype(np.int16))
    zc = (np.concatenate(zparts, axis=3)
          .reshape(P, tf).astype(np.float16))

    return {"tbl": const_cache["tbl"], "gidx": gidx, "cen": cen, "zc": zc,
            "wq": const_cache["wq"], "bnred": const_cache["bnred"],
            "bnbc": const_cache["bnbc"], "cbias": const_cache["cbias"]}


def assemble_output(results, orders=None):
    full = np.empty((8, 2, N, NFEAT), np.float32)             # [g, beta, n, f]
    for r in range(8):
        o = np.asarray(results[r]["out"]).reshape(8, 2, NSH, NFEAT)
        n0 = NSH * r
        if orders is None:
            full[:, :, n0:n0 + NSH, :] = o
        else:
            # position 32*nb + ns holds the atom with sorted rank 8*ns + nb
            full[:, :, n0 + _pos2atom(orders[r]), :] = o
    return full.reshape(B, N, NFEAT)


_cache = {}


def kernel(X, Nbrs, Nbrs_Z, rc, rs, re):
    from concourse.bass_utils import run_bass_kernel_spmd
    Nbrs_Z = np.asarray(Nbrs_Z)
    plan = compute_plan(Nbrs_Z)
    mps = plan[0]
    key = (tuple(np.asarray(rc).ravel().tolist()),
           tuple(np.asarray(rs).ravel().tolist()),
           tuple(np.asarray(re).ravel().tolist()), tuple(mps))
    if key not in _cache:
        _cache[key] = build_nc(np.asarray(rc).ravel(), np.asarray(rs).ravel(),
                               np.asarray(re).ravel(), mps=mps)
    nc = _cache[key]
    X = np.asarray(X, np.float32)
    Nbrs = np.asarray(Nbrs)
    cc = {"plan": plan}
    in_maps = [prep_core_inputs(X, Nbrs, Nbrs_Z, r, cc) for r in range(8)]
    cbias = make_cbias(np.asarray(rs).ravel(), np.asarray(re).ravel())
    for im in in_maps:
        im["cbias"] = cbias
    res = run_bass_kernel_spmd(nc, in_maps, core_ids=list(range(8)))
    return assemble_output(res.results, plan[1])

